# revision 11
# baseline (speedup 1.0000x reference)
"""Trainium2 Bass kernel for nn_Basic3DBlock (sparse 3D conv + sync BN + ReLU).

Dense-grid reformulation (8 NeuronCores, SPMD):
  The host reconstructs voxel grid coordinates from neighbor_idx (BFS over
  the 26-neighbor graph; fully verified against neighbor_idx), scatters
  features into a dense channel-major table [17, cells] (bf16; channel 16
  is the occupancy mask), and the device runs a dense 3x3x3 conv:

  - cells = x*H + y*Gz + z, sharded contiguously across 8 cores (+halo)
  - per tile of L=2046 cells: 3 window DMAs (one per dx) load X
    [51 = 3dx x 17ch, PW + 2*Gz]; dy and dz shifts live in the free dim
  - 12 PSUM-accumulated matmuls (3 dy offsets x 4 banks) produce
    psum [112, 2048] = 3 dz-variant blocks (partitions 0/32/64) + mask
    replicas (96) at shifted positions
  - vector combines the dz slices straight from PSUM (one PSUM operand
    per op, cross-partition-base PSUM+SB is allowed), applies the mask
  - scalar engine accumulates BN partial sums via activation accum_out
  - sync-BN: per-tile partial sums reduced on host; a tiny phase-2
    program applies scale/shift + ReLU reading the slab folded to 128
    partitions via strided DMA.

  Host does only index/layout work (BFS, scatter, final gather); all
  numerics (conv, BN stats, normalize, ReLU) run on device.
"""

import contextlib
import ctypes
import os
import sys
import types

import numpy as np

sys.path.insert(0, "/opt/trn_rl_repo")

N_CORES = 8
C_IN = 16
C_OUT = 16
EPS = 1e-5

PW = 2048            # PSUM tile width (fp32 columns); multiple of 512
L_TILE = PW - 2      # output cells per tile
F2 = 16384           # phase-2 chunk width

_OFFS = [(dx, dy, dz) for dx in (-1, 0, 1) for dy in (-1, 0, 1) for dz in (-1, 0, 1)]


# ---------------------------------------------------------------------------
# NTFF profiling hook shim (the agent image lacks antenv.axon_hooks; without
# it run_bass_kernel_spmd(trace=True) raises ImportError under axon).
# ---------------------------------------------------------------------------
def _ensure_ntff_hook(so_path="/opt/axon/libaxon_pjrt.so"):
    try:
        from antenv.axon_hooks import get_axon_ntff_profile_hook  # noqa: F401
        return True
    except ImportError:
        pass

    hook = None
    try:
        lib = ctypes.CDLL(so_path)
        if hasattr(lib, "axon_start_nrt_profile"):
            lib.axon_start_nrt_profile.argtypes = [
                ctypes.POINTER(ctypes.c_int64),
                ctypes.c_size_t,
            ]
            lib.axon_start_nrt_profile.restype = ctypes.c_int64
            lib.axon_stop_nrt_profile.argtypes = [ctypes.c_char_p]
            lib.axon_stop_nrt_profile.restype = ctypes.c_int64

            @contextlib.contextmanager
            def _hook(output_dir, device_ids):
                import jax
                jax.devices()
                if device_ids:
                    ids = (ctypes.c_int64 * len(device_ids))(*device_ids)
                    rc = lib.axon_start_nrt_profile(ids, len(device_ids))
                else:
                    rc = lib.axon_start_nrt_profile(None, 0)
                if rc != 0:
                    raise RuntimeError(f"axon_start_nrt_profile rc={rc}")
                try:
                    yield
                finally:
                    rc = lib.axon_stop_nrt_profile(str(output_dir).encode())
                    if rc < 0:
                        raise RuntimeError(f"axon_stop_nrt_profile rc={rc}")

            hook = _hook
    except OSError:
        hook = None

    mod = types.ModuleType("antenv.axon_hooks")
    _state = {"hook": hook}
    mod.get_axon_ntff_profile_hook = lambda: _state["hook"]
    mod.set_axon_ntff_profile_hook = lambda h: _state.update(hook=h)
    sys.modules["antenv.axon_hooks"] = mod
    try:
        import antenv
        antenv.axon_hooks = mod
    except ImportError:
        pass
    return hook is not None


# ---------------------------------------------------------------------------
# Host: grid reconstruction from neighbor_idx
# ---------------------------------------------------------------------------
def _reconstruct(nbr, n, max_components=64):
    """Assign each voxel a dense-grid cell so that every neighbor_idx entry
    becomes a fixed cell offset.  Returns (plin, (X_tot, Gy, Gz, H), n_cells)
    or None if the graph is not an exact 3D-grid adjacency."""
    xs = np.zeros(n, np.int32)
    ys = np.zeros(n, np.int32)
    zs = np.zeros(n, np.int32)
    comp = np.full(n, -1, np.int32)
    visited = np.zeros(n, bool)
    ncomp = 0
    ptr = 0
    while True:
        while ptr < n and visited[ptr]:
            ptr += 1
        if ptr >= n:
            break
        if ncomp >= max_components:
            return None
        seed = ptr
        visited[seed] = True
        comp[seed] = ncomp
        frontier = np.array([seed], np.int64)
        while frontier.size:
            nxt = []
            for k in range(27):
                if k == 13:
                    continue
                dx, dy, dz = _OFFS[k]
                nb = nbr[k, frontier]
                m = nb != n
                if not m.any():
                    continue
                nbm = nb[m]
                srcm = frontier[m]
                new = ~visited[nbm]
                if not new.any():
                    continue
                nb2 = nbm[new]
                src2 = srcm[new]
                visited[nb2] = True
                comp[nb2] = ncomp
                xs[nb2] = xs[src2] + dx
                ys[nb2] = ys[src2] + dy
                zs[nb2] = zs[src2] + dz
                nxt.append(nb2.astype(np.int64))
            frontier = np.concatenate(nxt) if nxt else np.empty(0, np.int64)
        ncomp += 1

    wx = np.zeros(ncomp, np.int64)
    wy = np.zeros(ncomp, np.int64)
    wz = np.zeros(ncomp, np.int64)
    for c in range(ncomp):
        m = comp == c
        xs[m] -= xs[m].min()
        ys[m] -= ys[m].min()
        zs[m] -= zs[m].min()
        wx[c] = xs[m].max() + 1
        wy[c] = ys[m].max() + 1
        wz[c] = zs[m].max() + 1

    Gy = int(wy.max()) + 2
    Gz = int(wz.max()) + 2
    offs = np.zeros(ncomp, np.int64)
    cur = 1
    for c in range(ncomp):
        offs[c] = cur
        cur += wx[c] + 1
    X_tot = int(cur)
    H = Gy * Gz
    n_cells = X_tot * H

    plin = (xs.astype(np.int64) + offs[comp]) * H \
        + (ys.astype(np.int64) + 1) * Gz + (zs.astype(np.int64) + 1)

    # full verification: the dense grid must reproduce nbr exactly
    D = np.full(n_cells, n, np.int32)
    D[plin] = np.arange(n, dtype=np.int32)
    for k in range(27):
        dx, dy, dz = _OFFS[k]
        delta = dx * H + dy * Gz + dz
        if not np.array_equal(D[plin + delta], nbr[k]):
            return None
    return plin, (X_tot, Gy, Gz, H), n_cells


# ---------------------------------------------------------------------------
# Device programs
# ---------------------------------------------------------------------------
def _build_phase1(S, T8, H, Gz, WIN):
    import concourse.bacc as bacc
    import concourse.tile as tile
    import concourse.mybir as mybir
    from concourse.bass import AP

    bf16 = mybir.dt.bfloat16
    fp32 = mybir.dt.float32
    L = L_TILE
    NB = PW // 512
    XW = PW + 2 * Gz

    nc = bacc.Bacc("TRN2", target_bir_lowering=False, debug=False,
                   num_devices=N_CORES)
    win_d = nc.dram_tensor("win", [17, WIN], bf16, kind="ExternalInput")
    w1_d = nc.dram_tensor("w1", [51, 336], bf16, kind="ExternalInput")
    y_d = nc.dram_tensor("y", [16, S], bf16, kind="ExternalOutput")
    stat_d = nc.dram_tensor("stat", [16, 2 * T8], fp32,
                            kind="ExternalOutput")

    with tile.TileContext(nc) as tc:
        with (
            tc.tile_pool(name="res", bufs=1) as res,
            tc.tile_pool(name="wk", bufs=3) as wk,
            tc.tile_pool(name="yo", bufs=3) as yo,
            tc.tile_pool(name="ps", bufs=2, space="PSUM") as ps,
        ):
            w1_sb = res.tile([51, 336], bf16)
            stt = res.tile([16, 2 * T8], fp32)
            xr = res.tile([51, 8 * XW], bf16)
            nc.sync.dma_start(w1_sb[:], w1_d[:])

            for t in range(T8):
                rx = (t % 8) * XW
                x1 = xr[:, rx:rx + XW]
                for dx in range(3):
                    srcw = AP(tensor=win_d, offset=t * L + dx * H,
                              ap=[[WIN, 17], [1, XW]])
                    eng = nc.sync if dx == 0 else nc.scalar
                    eng.dma_start(x1[dx * 17:(dx + 1) * 17], srcw)

                psum = ps.tile([112, PW], fp32, tag="ps")
                for dy in range(3):
                    for j in range(NB):
                        nc.tensor.matmul(
                            psum[:, j * 512:(j + 1) * 512],
                            lhsT=w1_sb[:, dy * 112:(dy + 1) * 112],
                            rhs=x1[:, dy * Gz + j * 512:dy * Gz + (j + 1) * 512],
                            start=(dy == 0), stop=(dy == 2))

                c0 = wk.tile([16, L], bf16, tag="c0")
                t1 = wk.tile([16, L], bf16, tag="t1")
                t2 = wk.tile([16, L], bf16, tag="t2")
                y_m = yo.tile([16, L], bf16, tag="ym")
                sq = yo.tile([16, L], bf16, tag="sq")
                nc.scalar.copy(c0[:], psum[0:16, 0:L])
                nc.vector.tensor_tensor(out=t1[:], in0=c0[:],
                                        in1=psum[32:48, 1:L + 1],
                                        op=mybir.AluOpType.add)
                nc.vector.tensor_tensor(out=t2[:], in0=t1[:],
                                        in1=psum[64:80, 2:L + 2],
                                        op=mybir.AluOpType.add)
                nc.vector.tensor_tensor(out=y_m[:], in0=t2[:],
                                        in1=psum[96:112, 1:L + 1],
                                        op=mybir.AluOpType.mult)
                nc.scalar.activation(sq[:], y_m[:],
                                     mybir.ActivationFunctionType.Copy,
                                     accum_out=stt[:, 2 * t:2 * t + 1])
                nc.scalar.activation(sq[:], y_m[:],
                                     mybir.ActivationFunctionType.Square,
                                     accum_out=stt[:, 2 * t + 1:2 * t + 2])
                nc.sync.dma_start(y_d[:, t * L:t * L + L], y_m[:])

            nc.sync.dma_start(stat_d[:], stt[:])

    nc.compile()
    return nc


def _build_phase2(S):
    import concourse.bacc as bacc
    import concourse.tile as tile
    import concourse.mybir as mybir
    from concourse.bass import AP

    bf16 = mybir.dt.bfloat16
    fp32 = mybir.dt.float32
    S8 = S // 8

    nc = bacc.Bacc("TRN2", target_bir_lowering=False, debug=False,
                   num_devices=N_CORES)
    y_in = nc.dram_tensor("y", [16, S], bf16, kind="ExternalInput")
    ss_d = nc.dram_tensor("ss", [128, 2], fp32, kind="ExternalInput")
    o_d = nc.dram_tensor("o", [16, S], bf16, kind="ExternalOutput")

    with tile.TileContext(nc) as tc:
        with (
            tc.tile_pool(name="res", bufs=1) as res,
            tc.tile_pool(name="io", bufs=3) as io,
        ):
            ss_sb = res.tile([128, 2], fp32)
            nc.sync.dma_start(ss_sb[:], ss_d[:])
            n_ch = -(-S8 // F2)
            for j in range(n_ch):
                w = min(F2, S8 - j * F2)
                yt = io.tile([128, F2], bf16, tag="yt")
                ot = io.tile([128, F2], bf16, tag="ot")
                src = AP(tensor=y_in, offset=j * F2,
                         ap=[[S, 16], [S8, 8], [1, w]])
                dst = AP(tensor=o_d, offset=j * F2,
                         ap=[[S, 16], [S8, 8], [1, w]])
                nc.sync.dma_start(yt[:, :w], src)
                nc.scalar.activation(ot[:, :w], yt[:, :w],
                                     mybir.ActivationFunctionType.Relu,
                                     bias=ss_sb[:, 1:2], scale=ss_sb[:, 0:1])
                nc.scalar.dma_start(dst, ot[:, :w])

    nc.compile()
    return nc


_P1_CACHE = {}
_P2_CACHE = {}


def _dense_kernel(features, weights, gamma, beta, neighbor_idx, trace):
    import ml_dtypes
    from concourse.bass_utils import run_bass_kernel_spmd

    bf16 = ml_dtypes.bfloat16
    n = features.shape[0]

    rec = _reconstruct(neighbor_idx, n)
    if rec is None:
        return None
    plin, (X_tot, Gy, Gz, H), n_cells = rec

    L = L_TILE
    T8 = -(-n_cells // (N_CORES * L))
    T8 = -(-T8 // 4) * 4  # keep S divisible by 8
    S = T8 * L
    guard = H + Gz + 1
    WIN = S + 2 * guard

    total = 2 * guard + N_CORES * S
    dense = np.zeros((17, total), dtype=bf16)
    dense[0:16, guard + plin] = features.astype(bf16).T
    dense[16, guard + plin] = np.float32(1.0)

    # weights for the stacked matmul: X row r = dx*17 + ch; one lhsT per dy
    # (columns dy*112 + dz*32 + co); the mask channel feeds columns 96..111
    # of the center (dx=1, dy=1) tap only.
    Wl = np.zeros((51, 3, 112), np.float32)
    for i_dx in range(3):
        for ch in range(C_IN):
            r = i_dx * 17 + ch
            for i_dy in range(3):
                for i_dz in range(3):
                    k = i_dx * 9 + i_dy * 3 + i_dz
                    Wl[r, i_dy, i_dz * 32:i_dz * 32 + 16] = weights[k][ch]
    Wl[1 * 17 + 16, 1, 96:112] = 1.0
    w1 = np.ascontiguousarray(Wl.reshape(51, 336)).astype(bf16)

    key = (S, T8, H, Gz, WIN)
    if key not in _P1_CACHE:
        _P1_CACHE[key] = _build_phase1(S, T8, H, Gz, WIN)
    p1 = _P1_CACHE[key]

    in_maps = []
    for c in range(N_CORES):
        win = np.ascontiguousarray(dense[:, c * S:c * S + WIN])
        in_maps.append({"win": win, "w1": w1})
    res1 = run_bass_kernel_spmd(p1, in_maps, core_ids=list(range(N_CORES)),
                                trace=trace)
    total_ns = res1.exec_time_ns or 0

    ssum = np.zeros(16, np.float64)
    ssq = np.zeros(16, np.float64)
    for c in range(N_CORES):
        st = res1.results[c]["stat"].astype(np.float64).reshape(16, -1, 2)
        ssum += st[:, :, 0].sum(axis=1)
        ssq += st[:, :, 1].sum(axis=1)
    mean = ssum / n
    var = ssq / n - mean * mean
    scale = gamma.astype(np.float64) / np.sqrt(var + EPS)
    shift = beta.astype(np.float64) - mean * scale
    ss_rep = np.empty((128, 2), np.float32)
    ss_rep[:, 0] = np.repeat(scale.astype(np.float32), 8)
    ss_rep[:, 1] = np.repeat(shift.astype(np.float32), 8)

    if S not in _P2_CACHE:
        _P2_CACHE[S] = _build_phase2(S)
    p2 = _P2_CACHE[S]
    in_maps2 = [{"y": res1.results[c]["y"], "ss": ss_rep}
                for c in range(N_CORES)]
    res2 = run_bass_kernel_spmd(p2, in_maps2, core_ids=list(range(N_CORES)),
                                trace=trace)
    total_ns += res2.exec_time_ns or 0

    y_full = np.concatenate([res2.results[c]["o"] for c in range(N_CORES)],
                            axis=1)
    out = y_full[:, plin].T.astype(np.float32)

    if total_ns:
        print(f"HW exec time: {total_ns} ns")
    return out


def kernel(features, weights, gamma, beta, neighbor_idx):
    _ensure_ntff_hook()
    features = np.asarray(features, dtype=np.float32)
    weights = np.asarray(weights, dtype=np.float32)
    gamma = np.asarray(gamma, dtype=np.float32)
    beta = np.asarray(beta, dtype=np.float32)
    neighbor_idx = np.asarray(neighbor_idx, dtype=np.int32)

    trace = os.environ.get("KERNEL_TRACE", "1") == "1"
    out = _dense_kernel(features, weights, gamma, beta, neighbor_idx, trace)
    if out is None:
        raise RuntimeError(
            "neighbor_idx is not an exact 3D-grid adjacency; dense path "
            "cannot be used")
    return out
